# revision 7
# baseline (speedup 1.0000x reference)
"""Multi-head attention forward on 8 TRN2 NeuronCores.

Problem: B=2, L=2048, D=1024, H=16, Hd=64 MHA block:
    qkv = x @ w_qkv + b_qkv ; per-head softmax(q k^T / sqrt(Hd)) @ v ; o @ w_out + b_out

Sharding (tensor parallel over heads x batch):
  core c -> batch c//4, heads [4*(c%4), 4*(c%4)+4).
  Each core computes its 4 heads' attention for its batch and a partial
  out-projection (2048, 1024) in bf16. Host sums the 4 partials per batch
  in fp32 and adds b_out.

v3 design notes (per-core PE stream budget, 2.4 GHz, bf16 1 row/cycle,
fp8 DoubleRow 0.5 cycles/row):
  - q/k/v projections run as 3-chain hi/lo fp8 DoubleRow matmuls
    (x = x_hi + x_lo in e4m3, w = w_hi(e4m3) + w_lo(e5m2), dropping the
    lo*lo term; ~0.2% relative).  2x faster than bf16.  The 1/sqrt(Hd)
    scale is NOT folded into wq (that would push it into e4m3's denormal
    range); it is applied by the exp's scale argument instead.
  - scores and PV stay bf16: the PE streams <=1 psum column per cycle in
    every non-DoubleRow mode, and attention quality gates fp8 here (fp8
    q/k or p costs 4-6% output error vs the 2e-2 budget).
  - exp splits across ScalarE (exact exp -> bf16, bias/scale fused) and
    VectorE (one-pass Schraudolph: i16 = s*23.083 + 15881.1 bitcast to
    bf16; ~2% systematic, on a small key subset).  Softmax renorm
    cancels the shared exp(-2) bias.
  - v is augmented with a ones column so the PV matmul accumulates the
    softmax denominators for free.
  - out-projection bf16, emitted per 1024-token slice right after that
    slice's normalization so it overlaps the next attention slice; y is
    returned bf16 and the host reduces partials in fp32.
"""

from contextlib import ExitStack

import numpy as np

B, L, D = 2, 2048, 1024
H, HD = 16, 64
NCORES = 8
CORES_PER_BATCH = 4
H_C = H // CORES_PER_BATCH          # heads per core = 4
COLS = H_C * HD                     # qkv cols per core = 256
P = 128
NKTP = D // P // 2                  # 4 DoubleRow contraction tile-pairs over D
NKB = L // P                        # 16 key/token blocks of 128
NMB = COLS // P                     # 2 col-blocks of the per-core qkv slice
NDT = COLS // P                     # 2 contraction tiles over per-core o dims
NS2 = L // 1024                     # 2 1024-token slices
SCALE = 1.0 / np.sqrt(np.float32(HD))
EXP_BIAS = -2.0                     # exp(s + EXP_BIAS); cancels in softmax
LOG2E = 1.4426950408889634
# bf16-bit-trick exp on raw (unscaled) scores: i16 = s*A16 + B16, bitcast bf16
SCHRAUD_A16 = 128.0 * LOG2E * SCALE
SCHRAUD_B16 = 16256.0 + EXP_BIAS * 128.0 * LOG2E - 5.58

_NC_CACHE = None
LAST_RESULTS = None


def _build_nc():
    import os
    import concourse.bass as bass
    import concourse.tile as tile
    from concourse import bacc, mybir

    f32 = mybir.dt.float32
    bf16 = mybir.dt.bfloat16
    e4 = mybir.dt.float8e4
    e5 = mybir.dt.float8e5
    i16 = mybir.dt.int16
    Exp = mybir.ActivationFunctionType.Exp
    Identity = mybir.ActivationFunctionType.Identity
    DR = mybir.MatmulPerfMode.DoubleRow
    mult = mybir.AluOpType.mult
    add = mybir.AluOpType.add

    # score blocks whose exp runs on VectorE via the bf16 bit trick
    DVE_KBS = frozenset(
        int(t) for t in os.environ.get("KDVE", "3,8,13").split(",") if t != ""
    )

    nc = bacc.Bacc(None, target_bir_lowering=False)

    # x / weights arrive pre-split hi/lo in fp8 with DoubleRow k-tile
    # pairing: [ktp, P, 2, ...] = rows (2*ktp*128 + p, (2*ktp+1)*128 + p).
    xh_d = nc.declare_dram_parameter("xh", [NKTP, P, 2, L], e4, isOutput=False)
    xl_d = nc.declare_dram_parameter("xl", [NKTP, P, 2, L], e4, isOutput=False)
    w_d = {}
    for wname in ("wqh", "wql", "wkh", "wkl", "wvh", "wvl"):
        # lo residuals live in e5m2: w ~ 1/sqrt(D) residuals are denormal in e4m3
        wdt = e4 if wname.endswith("h") else e5
        w_d[wname] = nc.declare_dram_parameter(
            wname, [NKTP, P, 2, COLS], wdt, isOutput=False
        )
    bq_d = nc.declare_dram_parameter("bq", [NMB, P, 1], f32, isOutput=False)
    bk_d = nc.declare_dram_parameter("bk", [NMB, P, 1], f32, isOutput=False)
    bv_d = nc.declare_dram_parameter("bv", [1, COLS], bf16, isOutput=False)
    wo_d = nc.declare_dram_parameter("wo", [NDT, P, D], bf16, isOutput=False)
    y_d = nc.declare_dram_parameter("y", [L, D], bf16, isOutput=True)

    with tile.TileContext(nc) as tc, ExitStack() as ctx, nc.allow_low_precision(
        "fp8 hi/lo projections + bf16 attention; softmax renorm bounds error"
    ):
        consts = ctx.enter_context(tc.tile_pool(name="consts", bufs=1))
        xtp = ctx.enter_context(tc.tile_pool(name="xtp", bufs=NKTP))
        wp = ctx.enter_context(tc.tile_pool(name="wp", bufs=NKTP))
        bigs = ctx.enter_context(tc.tile_pool(name="bigs", bufs=1))
        pp = ctx.enter_context(tc.tile_pool(name="pp", bufs=6))
        yp = ctx.enter_context(tc.tile_pool(name="yp", bufs=3))
        smallp = ctx.enter_context(tc.tile_pool(name="smallp", bufs=2))
        drp = ctx.enter_context(tc.tile_pool(name="drp", bufs=4, space="DRAM"))
        # PSUM budget (8 banks): big (128x1024 f32, 2 banks) x2 bufs = 4,
        # po (65x1024, 2 banks) x2 bufs = 4.
        psum = ctx.enter_context(tc.tile_pool(name="psum", bufs=2, space="PSUM"))
        psum_o = ctx.enter_context(tc.tile_pool(name="psum_o", bufs=2, space="PSUM"))

        # ---- constants ----
        ones_f32 = consts.tile([1, P], f32, tag="ones_f32")
        nc.vector.memset(ones_f32[:], 1.0)
        ones_sb = consts.tile([1, P], bf16, tag="ones")
        nc.vector.tensor_copy(ones_sb[:], ones_f32[:])
        bias_a = consts.tile([P, 1], f32, tag="bias_a")
        nc.vector.memset(bias_a[:], EXP_BIAS)
        bq_sb = consts.tile([P, NMB], f32, tag="bq")
        bk_sb = consts.tile([P, NMB], f32, tag="bk")
        for mb in range(NMB):
            nc.sync.dma_start(out=bq_sb[:, mb : mb + 1], in_=bq_d[mb])
            nc.sync.dma_start(out=bk_sb[:, mb : mb + 1], in_=bk_d[mb])
        bv_sb = consts.tile([1, COLS], bf16, tag="bv")
        nc.sync.dma_start(out=bv_sb[:], in_=bv_d[:])

        # ---- stream in x hi/lo and weights ----
        xh_t = [xtp.tile([P, 2, L], e4, tag="xh", name=f"xh{i}") for i in range(NKTP)]
        xl_t = [xtp.tile([P, 2, L], e4, tag="xl", name=f"xl{i}") for i in range(NKTP)]
        w_t = {
            wname: [
                wp.tile([P, 2, COLS], e4 if wname.endswith("h") else e5,
                        tag=wname, name=f"{wname}{i}")
                for i in range(NKTP)
            ]
            for wname in ("wqh", "wql", "wkh", "wkl", "wvh", "wvl")
        }
        qs_engines = [nc.sync, nc.scalar, nc.gpsimd]
        qi = 0

        def dma_in(out, in_):
            nonlocal qi
            qs_engines[qi % len(qs_engines)].dma_start(out=out, in_=in_)
            qi += 1

        # k/q weights first, then x in quarters (hi before lo), v weights early
        for wname in ("wkh", "wkl", "wqh", "wql"):
            for ktp in range(NKTP):
                dma_in(w_t[wname][ktp][:], w_d[wname][ktp])
        for quarter in range(4):
            sl = slice(quarter * (L // 4), (quarter + 1) * (L // 4))
            for ktp in range(NKTP):
                dma_in(xh_t[ktp][:, :, sl], xh_d[ktp][:, :, sl])
            for ktp in range(NKTP):
                dma_in(xl_t[ktp][:, :, sl], xl_d[ktp][:, :, sl])
            if quarter == 1:
                for wname in ("wvh", "wvl"):
                    for ktp in range(NKTP):
                        dma_in(w_t[wname][ktp][:], w_d[wname][ktp])
        wo_t = [wp.tile([P, D], bf16, tag="wo", name=f"wo{i}", bufs=NDT) for i in range(NDT)]
        for dt_i in range(NDT):
            nc.sync.dma_start(out=wo_t[dt_i][:], in_=wo_d[dt_i])

        # ---- persistent intermediates ----
        # q^T/k^T: partition = qkv col within a 128-block, dims (col_block, token)
        qt_sb = bigs.tile([P, NMB, L], bf16, tag="qt")
        kt_sb = bigs.tile([P, NMB, L], bf16, tag="kt")
        # v natural + ones column: partition = token within block, (kblock, head, hd+1)
        vx_sb = bigs.tile([P, NKB, H_C, HD + 1], bf16, tag="vx")
        nc.vector.memset(vx_sb[:, :, :, HD : HD + 1], 1.0)
        # normalized attention output, transposed: partition = o-dim within a
        # 128-block, dims (dim_block, token)
        ot_sb = bigs.tile([P, NDT, L], bf16, tag="ot")

        # hi/lo chains: drop the lo*lo term (~1e-3 relative)
        CHAINS = (("h", "h"), ("h", "l"), ("l", "h"))
        x_of = {"h": xh_t, "l": xl_t}

        # ---- phase 1: qkv projection (3-chain fp8 DoubleRow) ----
        def project_qk(mb, wtag, b_sb, dst, ns):
            ps = psum.tile([P, 1024], f32, tag="big", name="ps_qk")
            for half in range(2):
                n_sl = slice(ns * 1024 + half * 512, ns * 1024 + half * 512 + 512)
                first = True
                for ci, (wc, xc) in enumerate(CHAINS):
                    for ktp in range(NKTP):
                        nc.tensor.matmul(
                            ps[:, half * 512 : half * 512 + 512],
                            lhsT=w_t[wtag + wc][ktp][:, :, mb * P : (mb + 1) * P],
                            rhs=x_of[xc][ktp][:, :, n_sl],
                            start=first,
                            stop=(ci == len(CHAINS) - 1 and ktp == NKTP - 1),
                            perf_mode=DR,
                        )
                        first = False
            # bias-add + downcast on ScalarE (ACT is idle during projections)
            nc.scalar.activation(
                dst[:, mb, ns * 1024 : (ns + 1) * 1024], ps, Identity,
                bias=b_sb[:, mb : mb + 1],
            )

        def project_v(tb_range):
            for tb in tb_range:
                ps = psum.tile([P, 1024], f32, tag="big", name="ps_v")[:, :COLS]
                first = True
                for wc, xc in CHAINS:
                    for ktp in range(NKTP):
                        nc.tensor.matmul(
                            ps,
                            lhsT=x_of[xc][ktp][:, :, tb * P : (tb + 1) * P],
                            rhs=w_t["wv" + wc][ktp][:],
                            start=first,
                            stop=False,
                            perf_mode=DR,
                        )
                        first = False
                # rank-1 bias add: ones(128) x b_v(256), closes the psum group
                nc.tensor.matmul(
                    ps, lhsT=ones_sb[:], rhs=bv_sb[:], start=False, stop=True
                )
                nc.vector.tensor_copy(
                    vx_sb[:, tb, :, 0:HD],
                    ps.rearrange("p (h d) -> p h d", h=H_C),
                )

        # ---- phase 2: attention (scores^T -> exp -> PV w/ augmented v) ----
        def out_proj(tb, y_eng):
            ps = psum.tile([P, 1024], f32, tag="big", name="ps_y")
            for nb in range(2):
                for dt_i in range(NDT):
                    nc.tensor.matmul(
                        ps[:, nb * 512 : nb * 512 + 512],
                        lhsT=ot_sb[:, dt_i, tb * P : (tb + 1) * P],
                        rhs=wo_t[dt_i][:, nb * 512 : (nb + 1) * 512],
                        start=(dt_i == 0),
                        stop=(dt_i == NDT - 1),
                    )
            y_sb = yp.tile([P, 1024], bf16, tag="y", name="y_sb")
            if y_eng == 0:
                nc.scalar.copy(y_sb[:], ps)
            else:
                nc.vector.tensor_copy(y_sb[:], ps)
            nc.gpsimd.dma_start(out=y_d[tb * P : (tb + 1) * P, :], in_=y_sb[:])

        def attention(mb, qs):
            qsl = slice(qs * 1024, (qs + 1) * 1024)
            po = [
                psum_o.tile([HD + 1, 1024], f32, tag="po", name=f"po{hh}")
                for hh in range(2)
            ]
            # software-pipeline: pv runs PVLAG k-blocks behind the score/exp
            # stream so the PE has score work queued while normalization
            # drains po
            PVLAG = 2
            pending = []

            def emit_pv(kb, p2):
                for hh in range(2):
                    for half in range(2):
                        nc.tensor.matmul(
                            po[hh][:, half * 512 : half * 512 + 512],
                            lhsT=vx_sb[:, kb, 2 * mb + hh, :],
                            rhs=p2[hh][:, half * 512 : half * 512 + 512],
                            start=(kb == 0),
                            stop=(kb == NKB - 1),
                        )

            for kb in range(NKB):
                on_dve = kb in DVE_KBS
                p2 = []
                for hh in range(2):
                    off = hh * HD
                    ps = psum.tile([P, 1024], f32, tag="big", name=f"ps_s{hh}")
                    for half in range(2):
                        nc.tensor.matmul(
                            ps[:, half * 512 : half * 512 + 512],
                            lhsT=kt_sb[off : off + HD, mb, kb * P : (kb + 1) * P],
                            rhs=qt_sb[
                                off : off + HD,
                                mb,
                                qs * 1024 + half * 512 : qs * 1024 + half * 512 + 512,
                            ],
                            start=True,
                            stop=True,
                        )
                    p_sb = pp.tile([P, 1024], bf16, tag="p", name=f"p_sb{hh}")
                    if on_dve:
                        nc.vector.tensor_scalar(
                            p_sb[:].bitcast(i16), ps,
                            float(SCHRAUD_A16), float(SCHRAUD_B16), mult, add,
                        )
                    else:
                        nc.scalar.activation(
                            p_sb[:], ps, Exp, bias=bias_a[:], scale=float(SCALE)
                        )
                    p2.append(p_sb)
                pending.append((kb, p2))
                if len(pending) > PVLAG:
                    emit_pv(*pending.pop(0))
            for kb_p, p2_p in pending:
                emit_pv(kb_p, p2_p)

            # normalize: columns of po[0:HD] scaled by 1 / po[HD]
            for hh in range(2):
                off = hh * HD
                sums_sb = smallp.tile([1, 1024], f32, tag="sums", name="sums_sb")
                nc.vector.tensor_copy(sums_sb[:], po[hh][HD : HD + 1, :])
                # copy o out of PSUM immediately (bf16) so the po slot frees
                # for the next iteration's PV accumulation
                o_sb = smallp.tile([HD, 1024], bf16, tag="o_sb", name="o_sb", bufs=2)
                nc.vector.tensor_copy(o_sb[:], po[hh][0:HD, :])
                rec = smallp.tile([1, 1024], f32, tag="rec", name="rec")
                nc.vector.reciprocal_approx_fast(rec[:], sums_sb[:])
                rec_bf = smallp.tile([1, 1024], bf16, tag="rec_bf", name="rec_bf")
                nc.vector.tensor_copy(rec_bf[:], rec[:])
                # partition-broadcast via a DRAM bounce (zero-stride SBUF
                # APs are rejected; DRAM sources may broadcast)
                rec_dr = drp.tile([1, 1024], bf16, tag="rec_dr", name="rec_dr")
                nc.sync.dma_start(out=rec_dr[:], in_=rec_bf[:])
                pb_sb = smallp.tile([HD, 1024], bf16, tag="pb_sb", name="pb_sb")
                dr_ap = rec_dr[:]
                rec_bcast = bass.AP(
                    tensor=dr_ap.tensor,
                    offset=dr_ap.offset,
                    ap=[[0, HD], dr_ap.ap[-1]],
                )
                nc.sync.dma_start(out=pb_sb[:], in_=rec_bcast)
                nc.vector.tensor_tensor(
                    ot_sb[off : off + HD, mb, qsl], o_sb[:], pb_sb[:], mult
                )

        # ---- schedule ----
        for ns in range(NS2):
            project_qk(0, "wk", bk_sb, kt_sb, ns)
        project_qk(0, "wq", bq_sb, qt_sb, 0)
        project_v(range(NKB))
        project_qk(0, "wq", bq_sb, qt_sb, 1)
        for mb in range(NMB):
            if mb == 1:
                for ns in range(NS2):
                    project_qk(1, "wq", bq_sb, qt_sb, ns)
                    project_qk(1, "wk", bk_sb, kt_sb, ns)
            for qs in range(NS2):
                attention(mb, qs)
                if mb == 1:
                    for i, tb in enumerate(range(qs * 8, qs * 8 + 8)):
                        out_proj(tb, y_eng=i % 2)

    nc.finalize()
    return nc


def get_nc():
    global _NC_CACHE
    if _NC_CACHE is None:
        _NC_CACHE = _build_nc()
    return _NC_CACHE


def _hi_lo(a, hidt, lodt):
    hi = a.astype(hidt)
    lo = (a - hi.astype(np.float32)).astype(lodt)
    return hi, lo


def make_in_maps(x, w_qkv, b_qkv, w_out):
    import ml_dtypes

    bf16 = ml_dtypes.bfloat16
    e4 = ml_dtypes.float8_e4m3
    e5 = ml_dtypes.float8_e5m2
    x = np.asarray(x, dtype=np.float32)
    w_qkv = np.asarray(w_qkv, dtype=np.float32)
    b_qkv = np.asarray(b_qkv, dtype=np.float32)
    w_out = np.asarray(w_out, dtype=np.float32)

    def pair_kt(a):
        # (D, cols) -> [NKTP, P, 2, cols] with DoubleRow k-tile pairing
        cols = a.shape[1]
        return np.ascontiguousarray(
            a.reshape(NKTP, 2, P, cols).transpose(0, 2, 1, 3)
        )

    in_maps = []
    for c in range(NCORES):
        b, g = divmod(c, CORES_PER_BATCH)
        cs, ce = g * COLS, (g + 1) * COLS
        xt = pair_kt(np.ascontiguousarray(x[b].T))          # [4, 128, 2, L]
        xh, xl = _hi_lo(xt, e4, e4)
        wq = pair_kt(w_qkv[:, 0 * D : 1 * D][:, cs:ce])
        wk = pair_kt(w_qkv[:, 1 * D : 2 * D][:, cs:ce])
        wv = pair_kt(w_qkv[:, 2 * D : 3 * D][:, cs:ce])
        wqh, wql = _hi_lo(wq, e4, e5)
        wkh, wkl = _hi_lo(wk, e4, e5)
        wvh, wvl = _hi_lo(wv, e4, e5)
        bq = np.ascontiguousarray(b_qkv[0 * D : 1 * D][cs:ce]).reshape(NMB, P, 1)
        bk = np.ascontiguousarray(b_qkv[1 * D : 2 * D][cs:ce]).reshape(NMB, P, 1)
        bv = b_qkv[2 * D : 3 * D][cs:ce].reshape(1, COLS).astype(bf16)
        wo = np.ascontiguousarray(w_out[cs:ce, :]).reshape(NDT, P, D).astype(bf16)
        in_maps.append(
            dict(xh=xh, xl=xl, wqh=wqh, wql=wql, wkh=wkh, wkl=wkl,
                 wvh=wvh, wvl=wvl, bq=bq, bk=bk, bv=bv, wo=wo)
        )
    return in_maps


def kernel(x, w_qkv, b_qkv, w_out, b_out, _trace=False, **_kw):
    global LAST_RESULTS
    from concourse.bass_utils import run_bass_kernel_spmd

    nc = get_nc()
    in_maps = make_in_maps(x, w_qkv, b_qkv, w_out)
    res = run_bass_kernel_spmd(nc, in_maps, list(range(NCORES)), trace=_trace, **_kw)
    LAST_RESULTS = res

    b_out = np.asarray(b_out, dtype=np.float32)
    y = np.zeros((B, L, D), dtype=np.float32)
    for c in range(NCORES):
        y[c // CORES_PER_BATCH] += res.results[c]["y"].astype(np.float32)
    y += b_out[None, None, :]
    return y


# revision 9
# speedup vs baseline: 1.2317x; 1.2317x over previous
"""Multi-head attention forward on 8 TRN2 NeuronCores.

Problem: B=2, L=2048, D=1024, H=16, Hd=64 MHA block:
    qkv = x @ w_qkv + b_qkv ; per-head softmax(q k^T / sqrt(Hd)) @ v ; o @ w_out + b_out

Sharding (tensor parallel over heads x batch):
  core c -> batch c//4, heads [4*(c%4), 4*(c%4)+4).
  Each core computes its 4 heads' attention for its batch and a partial
  out-projection (2048, 1024) in bf16. Host sums the 4 partials per batch
  in fp32 and adds b_out.

v4 design notes.  Measured TRN2 facts driving this schedule:
  - The PE streams exactly one psum column per 2.4GHz cycle in every
    mode (bf16/fp8/DoubleRow all measured 216ns serialized for N=512),
    so the bf16 matmul plan below is already at the column floor
    (~394k columns/core).  fp8 DoubleRow halves contraction passes only
    with both operands single-fp8, which costs 2.5-5% output error --
    over the 2e-2 budget.  Everything stays bf16.
  - The HAM activity throttle caps sustained PE duty (~74% long-run,
    50%-duty windows after ~60-90us of saturation); PE idle gaps also
    cost a 1.2GHz p-state ramp on resume.  So the schedule's job is a
    gap-free in-order PE queue, not fewer flops: projection work and
    the out-projection are interleaved into the attention loop as
    "filler" units, one per key block, and the tail is kept minimal.
  - exp splits across ScalarE (exact exp -> bf16) and VectorE
    (one-pass Schraudolph: i16 = s*184.66 + 16250.4 bitcast bf16, ~2%
    systematic, on 3 of 16 key blocks) so neither engine gates the
    attention loop (ACT alone would need 133us > attention PE time).
  - v's bias is added during the psum->SBUF copy against a
    host-broadcast [128,256] bias tile (kills 16 rank-1 PE matmuls);
    v carries a ones column so PV accumulates softmax denominators.
  - y is written bf16 (halves the output DMA) on the GpSimd queue;
    psum->y copies alternate ScalarE/VectorE.
"""

from collections import deque
from contextlib import ExitStack

import numpy as np

B, L, D = 2, 2048, 1024
H, HD = 16, 64
NCORES = 8
CORES_PER_BATCH = 4
H_C = H // CORES_PER_BATCH          # heads per core = 4
COLS = H_C * HD                     # qkv cols per core = 256
P = 128
NKT = D // P                        # 8 contraction tiles over D
NKB = L // P                        # 16 key/token blocks of 128
NMB = COLS // P                     # 2 col-blocks of the per-core qkv slice
NDT = COLS // P                     # 2 contraction tiles over per-core o dims
NS2 = L // 1024                     # 2 1024-token slices
SCALE = 1.0 / np.sqrt(np.float32(HD))
LOG2E = 1.4426950408889634
# bf16-bit-trick exp on (pre-scaled) scores: i16 = s*A16 + B16, bitcast bf16
SCHRAUD_A16 = 128.0 * LOG2E
SCHRAUD_B16 = 16256.0 - 5.58

_NC_CACHE = None
LAST_RESULTS = None


def _build_nc():
    import os
    import concourse.bass as bass
    import concourse.tile as tile
    from concourse import bacc, mybir

    f32 = mybir.dt.float32
    bf16 = mybir.dt.bfloat16
    i16 = mybir.dt.int16
    Exp = mybir.ActivationFunctionType.Exp
    mult = mybir.AluOpType.mult
    add = mybir.AluOpType.add

    # score blocks whose exp runs on VectorE via the bf16 bit trick
    DVE_KBS = frozenset(
        int(t) for t in os.environ.get("KDVE", "3,8,13").split(",") if t != ""
    )

    nc = bacc.Bacc(None, target_bir_lowering=False)

    xt_d = nc.declare_dram_parameter("xt", [NKT, P, L], bf16, isOutput=False)
    wq_d = nc.declare_dram_parameter("wq", [NKT, P, COLS], bf16, isOutput=False)
    wk_d = nc.declare_dram_parameter("wk", [NKT, P, COLS], bf16, isOutput=False)
    wv_d = nc.declare_dram_parameter("wv", [NKT, P, COLS], bf16, isOutput=False)
    bq_d = nc.declare_dram_parameter("bq", [NMB, P, 1], f32, isOutput=False)
    bk_d = nc.declare_dram_parameter("bk", [NMB, P, 1], f32, isOutput=False)
    bvb_d = nc.declare_dram_parameter("bvb", [P, COLS], f32, isOutput=False)
    wo_d = nc.declare_dram_parameter("wo", [NDT, P, D], bf16, isOutput=False)
    y_d = nc.declare_dram_parameter("y", [L, D], bf16, isOutput=True)

    with tile.TileContext(nc) as tc, ExitStack() as ctx, nc.allow_low_precision(
        "bf16 matmul operands; accumulation stays fp32 in PSUM"
    ):
        consts = ctx.enter_context(tc.tile_pool(name="consts", bufs=1))
        xtp = ctx.enter_context(tc.tile_pool(name="xtp", bufs=NKT))
        wp = ctx.enter_context(tc.tile_pool(name="wp", bufs=NKT))
        bigs = ctx.enter_context(tc.tile_pool(name="bigs", bufs=1))
        pp = ctx.enter_context(tc.tile_pool(name="pp", bufs=6))
        yp = ctx.enter_context(tc.tile_pool(name="yp", bufs=3))
        smallp = ctx.enter_context(tc.tile_pool(name="smallp", bufs=2))
        drp = ctx.enter_context(tc.tile_pool(name="drp", bufs=4, space="DRAM"))
        # PSUM budget (8 banks): big (128x1024 f32, 2 banks) x2 bufs = 4,
        # po (65x1024, 2 banks) x2 bufs = 4.
        psum = ctx.enter_context(tc.tile_pool(name="psum", bufs=2, space="PSUM"))
        psum_o = ctx.enter_context(tc.tile_pool(name="psum_o", bufs=2, space="PSUM"))

        # ---- constants ----
        bq_sb = consts.tile([P, NMB], f32, tag="bq")
        bk_sb = consts.tile([P, NMB], f32, tag="bk")
        for mb in range(NMB):
            nc.sync.dma_start(out=bq_sb[:, mb : mb + 1], in_=bq_d[mb])
            nc.sync.dma_start(out=bk_sb[:, mb : mb + 1], in_=bk_d[mb])
        bvb_sb = consts.tile([P, COLS], f32, tag="bvb")
        nc.sync.dma_start(out=bvb_sb[:], in_=bvb_d[:])

        # ---- stream in x^T and weights as per-k-tile tiles ----
        xt_t = [xtp.tile([P, L], bf16, tag="xt", name=f"xt{i}") for i in range(NKT)]
        wq_t = [wp.tile([P, COLS], bf16, tag="wq", name=f"wq{i}") for i in range(NKT)]
        wk_t = [wp.tile([P, COLS], bf16, tag="wk", name=f"wk{i}") for i in range(NKT)]
        wv_t = [wp.tile([P, COLS], bf16, tag="wv", name=f"wv{i}") for i in range(NKT)]
        qs_engines = [nc.sync, nc.scalar, nc.gpsimd]
        qi = 0

        def dma_in(out, in_):
            nonlocal qi
            qs_engines[qi % len(qs_engines)].dma_start(out=out, in_=in_)
            qi += 1

        # k/q weights first, x quarters next (attention on head pair 0 starts
        # after k(ns0)+q(ns0)), v weights at quarter 1
        for kt in range(NKT):
            dma_in(wk_t[kt][:], wk_d[kt])
            dma_in(wq_t[kt][:], wq_d[kt])
        for quarter in range(4):
            sl = slice(quarter * (L // 4), (quarter + 1) * (L // 4))
            for kt in range(NKT):
                dma_in(xt_t[kt][:, sl], xt_d[kt][:, sl])
            if quarter == 1:
                for kt in range(NKT):
                    dma_in(wv_t[kt][:], wv_d[kt])
        wo_t = [wp.tile([P, D], bf16, tag="wo", name=f"wo{i}", bufs=NDT) for i in range(NDT)]
        for dt_i in range(NDT):
            nc.sync.dma_start(out=wo_t[dt_i][:], in_=wo_d[dt_i])

        # ---- persistent intermediates ----
        # q^T/k^T: partition = qkv col within a 128-block, dims (col_block, token)
        qt_sb = bigs.tile([P, NMB, L], bf16, tag="qt")
        kt_sb = bigs.tile([P, NMB, L], bf16, tag="kt")
        # v natural + ones column: partition = token within block, (kblock, head, hd+1)
        vx_sb = bigs.tile([P, NKB, H_C, HD + 1], bf16, tag="vx")
        nc.vector.memset(vx_sb[:, :, :, HD : HD + 1], 1.0)
        # normalized attention output, transposed: partition = o-dim within a
        # 128-block, dims (dim_block, token)
        ot_sb = bigs.tile([P, NDT, L], bf16, tag="ot")

        # ---- building blocks ----
        def project_qk(mb, w_t_, b_sb, dst, ns):
            ps = psum.tile([P, 1024], f32, tag="big", name="ps_qk")
            for half in range(2):
                for kt in range(NKT):
                    nc.tensor.matmul(
                        ps[:, half * 512 : half * 512 + 512],
                        lhsT=w_t_[kt][:, mb * P : (mb + 1) * P],
                        rhs=xt_t[kt][
                            :, ns * 1024 + half * 512 : ns * 1024 + half * 512 + 512
                        ],
                        start=(kt == 0),
                        stop=(kt == NKT - 1),
                    )
            nc.vector.tensor_scalar_add(
                dst[:, mb, ns * 1024 : (ns + 1) * 1024], ps, b_sb[:, mb : mb + 1]
            )

        def project_v(tb):
            ps = psum.tile([P, 1024], f32, tag="big", name="ps_v")[:, :COLS]
            for kt in range(NKT):
                nc.tensor.matmul(
                    ps,
                    lhsT=xt_t[kt][:, tb * P : (tb + 1) * P],
                    rhs=wv_t[kt][:],
                    start=(kt == 0),
                    stop=(kt == NKT - 1),
                )
            # bias add fused into the psum->SBUF copy (host-broadcast bias)
            nc.vector.tensor_tensor(
                vx_sb[:, tb, :, 0:HD],
                ps.rearrange("p (h d) -> p h d", h=H_C),
                bvb_sb.rearrange("p (h d) -> p h d", h=H_C),
                add,
            )

        def out_proj(tb, y_eng):
            ps = psum.tile([P, 1024], f32, tag="big", name="ps_y")
            for nb in range(2):
                for dt_i in range(NDT):
                    nc.tensor.matmul(
                        ps[:, nb * 512 : nb * 512 + 512],
                        lhsT=ot_sb[:, dt_i, tb * P : (tb + 1) * P],
                        rhs=wo_t[dt_i][:, nb * 512 : (nb + 1) * 512],
                        start=(dt_i == 0),
                        stop=(dt_i == NDT - 1),
                    )
            y_sb = yp.tile([P, 1024], bf16, tag="y", name="y_sb")
            if y_eng == 0:
                nc.scalar.copy(y_sb[:], ps)
            else:
                nc.vector.tensor_copy(y_sb[:], ps)
            nc.gpsimd.dma_start(out=y_d[tb * P : (tb + 1) * P, :], in_=y_sb[:])

        # ---- attention with filler interleaving ----
        # Per key block: scores^T (bf16, K=64, head pairs at PE rows 0/64)
        # -> exp (ACT exact / DVE Schraudolph) -> PV with augmented v,
        # software-pipelined PVLAG blocks behind the score stream.  One
        # filler unit (a projection slice / out_proj) is emitted per key
        # block so the in-order PE queue never drains.
        def attention(mb, qs, filler):
            qsl = slice(qs * 1024, (qs + 1) * 1024)
            po = [
                psum_o.tile([HD + 1, 1024], f32, tag="po", name=f"po{hh}")
                for hh in range(2)
            ]
            PVLAG = 2
            pending = []

            def emit_pv(kb, p2):
                for hh in range(2):
                    for half in range(2):
                        nc.tensor.matmul(
                            po[hh][:, half * 512 : half * 512 + 512],
                            lhsT=vx_sb[:, kb, 2 * mb + hh, :],
                            rhs=p2[hh][:, half * 512 : half * 512 + 512],
                            start=(kb == 0),
                            stop=(kb == NKB - 1),
                        )

            for kb in range(NKB):
                if filler:
                    filler.popleft()()
                on_dve = kb in DVE_KBS
                p2 = []
                for hh in range(2):
                    off = hh * HD
                    ps = psum.tile([P, 1024], f32, tag="big", name=f"ps_s{hh}")
                    for half in range(2):
                        nc.tensor.matmul(
                            ps[:, half * 512 : half * 512 + 512],
                            lhsT=kt_sb[off : off + HD, mb, kb * P : (kb + 1) * P],
                            rhs=qt_sb[
                                off : off + HD,
                                mb,
                                qs * 1024 + half * 512 : qs * 1024 + half * 512 + 512,
                            ],
                            start=True,
                            stop=True,
                        )
                    p_sb = pp.tile([P, 1024], bf16, tag="p", name=f"p_sb{hh}")
                    if on_dve:
                        nc.vector.tensor_scalar(
                            p_sb[:].bitcast(i16), ps,
                            float(SCHRAUD_A16), float(SCHRAUD_B16), mult, add,
                        )
                    else:
                        nc.scalar.activation(p_sb[:], ps, Exp)
                    p2.append(p_sb)
                pending.append((kb, p2))
                if len(pending) > PVLAG:
                    emit_pv(*pending.pop(0))
            for kb_p, p2_p in pending:
                emit_pv(kb_p, p2_p)

            # normalize: columns of po[0:HD] scaled by 1 / po[HD]
            for hh in range(2):
                off = hh * HD
                sums_sb = smallp.tile([1, 1024], f32, tag="sums", name="sums_sb")
                nc.vector.tensor_copy(sums_sb[:], po[hh][HD : HD + 1, :])
                # copy o out of PSUM immediately (bf16) so the po slot frees
                # for the next iteration's PV accumulation
                o_sb = smallp.tile([HD, 1024], bf16, tag="o_sb", name="o_sb", bufs=2)
                nc.vector.tensor_copy(o_sb[:], po[hh][0:HD, :])
                rec = smallp.tile([1, 1024], f32, tag="rec", name="rec")
                nc.vector.reciprocal_approx_fast(rec[:], sums_sb[:])
                rec_bf = smallp.tile([1, 1024], bf16, tag="rec_bf", name="rec_bf")
                nc.vector.tensor_copy(rec_bf[:], rec[:])
                # partition-broadcast via a DRAM bounce (zero-stride SBUF
                # APs are rejected; DRAM sources may broadcast)
                rec_dr = drp.tile([1, 1024], bf16, tag="rec_dr", name="rec_dr")
                nc.sync.dma_start(out=rec_dr[:], in_=rec_bf[:])
                pb_sb = smallp.tile([HD, 1024], bf16, tag="pb_sb", name="pb_sb")
                dr_ap = rec_dr[:]
                rec_bcast = bass.AP(
                    tensor=dr_ap.tensor,
                    offset=dr_ap.offset,
                    ap=[[0, HD], dr_ap.ap[-1]],
                )
                nc.sync.dma_start(out=pb_sb[:], in_=rec_bcast)
                nc.vector.tensor_tensor(
                    ot_sb[off : off + HD, mb, qsl], o_sb[:], pb_sb[:], mult
                )

        # ---- schedule ----
        # Minimal prologue: k/q/v for the first 1024-token slice of col-block
        # 0, then attention(0,0) starts while fillers stream the rest of the
        # projections through the attention loop's PE slack.
        project_qk(0, wk_t, bk_sb, kt_sb, 0)
        project_qk(0, wq_t, bq_sb, qt_sb, 0)
        for tb in range(4):
            project_v(tb)

        f00 = deque([
            (lambda t: lambda: project_v(t))(tb) for tb in range(4, 10)
        ])
        f00.insert(2, lambda: project_qk(0, wk_t, bk_sb, kt_sb, 1))
        f00.extend((lambda t: lambda: project_v(t))(tb) for tb in range(10, 16))
        f00.append(lambda: project_qk(0, wq_t, bq_sb, qt_sb, 1))
        attention(0, 0, f00)

        f01 = deque([
            lambda: project_qk(1, wq_t, bq_sb, qt_sb, 0),
            lambda: project_qk(1, wk_t, bk_sb, kt_sb, 0),
            lambda: project_qk(1, wq_t, bq_sb, qt_sb, 1),
            lambda: project_qk(1, wk_t, bk_sb, kt_sb, 1),
        ])
        attention(0, 1, f01)
        attention(1, 0, deque())
        f11 = deque([lambda: None] * 3)
        for i, tb in enumerate(range(0, 8)):
            f11.append((lambda t, e: lambda: out_proj(t, e))(tb, i % 2))
        attention(1, 1, f11)
        for i, tb in enumerate(range(8, 16)):
            out_proj(tb, y_eng=i % 2)

    nc.finalize()
    return nc


def get_nc():
    global _NC_CACHE
    if _NC_CACHE is None:
        _NC_CACHE = _build_nc()
    return _NC_CACHE


def make_in_maps(x, w_qkv, b_qkv, w_out):
    import ml_dtypes

    bf16 = ml_dtypes.bfloat16
    x = np.asarray(x, dtype=np.float32)
    w_qkv = np.asarray(w_qkv, dtype=np.float32)
    b_qkv = np.asarray(b_qkv, dtype=np.float32)
    w_out = np.asarray(w_out, dtype=np.float32)

    in_maps = []
    for c in range(NCORES):
        b, g = divmod(c, CORES_PER_BATCH)
        cs, ce = g * COLS, (g + 1) * COLS
        xt = np.ascontiguousarray(x[b].T).reshape(NKT, P, L).astype(bf16)
        wq = (w_qkv[:, 0 * D : 1 * D][:, cs:ce] * SCALE).reshape(NKT, P, COLS).astype(bf16)
        wk = np.ascontiguousarray(w_qkv[:, 1 * D : 2 * D][:, cs:ce]).reshape(NKT, P, COLS).astype(bf16)
        wv = np.ascontiguousarray(w_qkv[:, 2 * D : 3 * D][:, cs:ce]).reshape(NKT, P, COLS).astype(bf16)
        bq = np.ascontiguousarray(b_qkv[0 * D : 1 * D][cs:ce] * SCALE).reshape(
            NMB, P, 1
        )
        bk = np.ascontiguousarray(b_qkv[1 * D : 2 * D][cs:ce]).reshape(NMB, P, 1)
        bvb = np.broadcast_to(
            b_qkv[2 * D : 3 * D][cs:ce].astype(np.float32), (P, COLS)
        ).copy()
        wo = np.ascontiguousarray(w_out[cs:ce, :]).reshape(NDT, P, D).astype(bf16)
        in_maps.append(
            dict(xt=xt, wq=wq, wk=wk, wv=wv, bq=bq, bk=bk, bvb=bvb, wo=wo)
        )
    return in_maps


def kernel(x, w_qkv, b_qkv, w_out, b_out, _trace=False, **_kw):
    global LAST_RESULTS
    from concourse.bass_utils import run_bass_kernel_spmd

    nc = get_nc()
    in_maps = make_in_maps(x, w_qkv, b_qkv, w_out)
    res = run_bass_kernel_spmd(nc, in_maps, list(range(NCORES)), trace=_trace, **_kw)
    LAST_RESULTS = res

    b_out = np.asarray(b_out, dtype=np.float32)
    y = np.zeros((B, L, D), dtype=np.float32)
    for c in range(NCORES):
        y[c // CORES_PER_BATCH] += res.results[c]["y"].astype(np.float32)
    y += b_out[None, None, :]
    return y


# revision 12
# speedup vs baseline: 1.2326x; 1.0007x over previous
"""Multi-head attention forward on 8 TRN2 NeuronCores.

Problem: B=2, L=2048, D=1024, H=16, Hd=64 MHA block:
    qkv = x @ w_qkv + b_qkv ; per-head softmax(q k^T / sqrt(Hd)) @ v ; o @ w_out + b_out

Sharding (tensor parallel over heads x batch):
  core c -> batch c//4, heads [4*(c%4), 4*(c%4)+4).
  Each core computes its 4 heads' attention for its batch and a partial
  out-projection (2048, 1024) in bf16. Host sums the 4 partials per batch
  in fp32 and adds b_out.

v4 design notes.  Measured TRN2 facts driving this schedule:
  - The PE streams exactly one psum column per 2.4GHz cycle in every
    mode (bf16/fp8/DoubleRow all measured 216ns serialized for N=512),
    so the bf16 matmul plan below is already at the column floor
    (~394k columns/core).  fp8 DoubleRow halves contraction passes only
    with both operands single-fp8, which costs 2.5-5% output error --
    over the 2e-2 budget.  Everything stays bf16.
  - The HAM activity throttle caps sustained PE duty (~74% long-run,
    50%-duty windows after ~60-90us of saturation); PE idle gaps also
    cost a 1.2GHz p-state ramp on resume.  So the schedule's job is a
    gap-free in-order PE queue, not fewer flops: projection work and
    the out-projection are interleaved into the attention loop as
    "filler" units, one per key block, and the tail is kept minimal.
  - exp splits across ScalarE (exact exp -> bf16) and VectorE
    (one-pass Schraudolph: i16 = s*184.66 + 16250.4 bitcast bf16, ~2%
    systematic, on 3 of 16 key blocks) so neither engine gates the
    attention loop (ACT alone would need 133us > attention PE time).
  - v's bias is added during the psum->SBUF copy against a
    host-broadcast [128,256] bias tile (kills 16 rank-1 PE matmuls);
    v carries a ones column so PV accumulates softmax denominators.
  - y is written bf16 (halves the output DMA) on the GpSimd queue;
    psum->y copies alternate ScalarE/VectorE.
"""

from collections import deque
from contextlib import ExitStack

import numpy as np

B, L, D = 2, 2048, 1024
H, HD = 16, 64
NCORES = 8
CORES_PER_BATCH = 4
H_C = H // CORES_PER_BATCH          # heads per core = 4
COLS = H_C * HD                     # qkv cols per core = 256
P = 128
NKT = D // P                        # 8 contraction tiles over D
NKB = L // P                        # 16 key/token blocks of 128
NMB = COLS // P                     # 2 col-blocks of the per-core qkv slice
NDT = COLS // P                     # 2 contraction tiles over per-core o dims
NS2 = L // 1024                     # 2 1024-token slices
SCALE = 1.0 / np.sqrt(np.float32(HD))
LOG2E = 1.4426950408889634
# bf16-bit-trick exp on (pre-scaled) scores: i16 = s*A16 + B16, bitcast bf16
SCHRAUD_A16 = 128.0 * LOG2E
SCHRAUD_B16 = 16256.0 - 5.58

_NC_CACHE = None
LAST_RESULTS = None


def _build_nc():
    import os
    import concourse.bass as bass
    import concourse.tile as tile
    from concourse import bacc, mybir

    f32 = mybir.dt.float32
    bf16 = mybir.dt.bfloat16
    i16 = mybir.dt.int16
    Exp = mybir.ActivationFunctionType.Exp
    mult = mybir.AluOpType.mult
    add = mybir.AluOpType.add

    # score blocks whose exp runs on VectorE via the bf16 bit trick
    DVE_KBS = frozenset(
        int(t) for t in os.environ.get("KDVE", "3,8,13").split(",") if t != ""
    )

    nc = bacc.Bacc(None, target_bir_lowering=False)

    xt_d = nc.declare_dram_parameter("xt", [NKT, P, L], bf16, isOutput=False)
    wq_d = nc.declare_dram_parameter("wq", [NKT, P, COLS], bf16, isOutput=False)
    wk_d = nc.declare_dram_parameter("wk", [NKT, P, COLS], bf16, isOutput=False)
    wv_d = nc.declare_dram_parameter("wv", [NKT, P, COLS], bf16, isOutput=False)
    bq_d = nc.declare_dram_parameter("bq", [NMB, P, 1], f32, isOutput=False)
    bk_d = nc.declare_dram_parameter("bk", [NMB, P, 1], f32, isOutput=False)
    bvb_d = nc.declare_dram_parameter("bvb", [P, COLS], f32, isOutput=False)
    wo_d = nc.declare_dram_parameter("wo", [NDT, P, D], bf16, isOutput=False)
    y_d = nc.declare_dram_parameter("y", [L, D], bf16, isOutput=True)

    with tile.TileContext(nc) as tc, ExitStack() as ctx, nc.allow_low_precision(
        "bf16 matmul operands; accumulation stays fp32 in PSUM"
    ):
        consts = ctx.enter_context(tc.tile_pool(name="consts", bufs=1))
        xtp = ctx.enter_context(tc.tile_pool(name="xtp", bufs=NKT))
        wp = ctx.enter_context(tc.tile_pool(name="wp", bufs=NKT))
        bigs = ctx.enter_context(tc.tile_pool(name="bigs", bufs=1))
        pp = ctx.enter_context(tc.tile_pool(name="pp", bufs=6))
        yp = ctx.enter_context(tc.tile_pool(name="yp", bufs=3))
        smallp = ctx.enter_context(tc.tile_pool(name="smallp", bufs=2))
        drp = ctx.enter_context(tc.tile_pool(name="drp", bufs=4, space="DRAM"))
        # PSUM budget (8 banks): big (128x1024 f32, 2 banks) x2 bufs = 4,
        # po (65x1024, 2 banks) x2 bufs = 4.
        psum = ctx.enter_context(tc.tile_pool(name="psum", bufs=2, space="PSUM"))
        psum_o = ctx.enter_context(tc.tile_pool(name="psum_o", bufs=2, space="PSUM"))

        # ---- constants ----
        bq_sb = consts.tile([P, NMB], f32, tag="bq")
        bk_sb = consts.tile([P, NMB], f32, tag="bk")
        bvb_sb = consts.tile([P, COLS], f32, tag="bvb")

        # ---- stream in x^T and weights as per-k-tile tiles ----
        xt_t = [xtp.tile([P, L], bf16, tag="xt", name=f"xt{i}") for i in range(NKT)]
        wq_t = [wp.tile([P, COLS], bf16, tag="wq", name=f"wq{i}") for i in range(NKT)]
        wk_t = [wp.tile([P, COLS], bf16, tag="wk", name=f"wk{i}") for i in range(NKT)]
        wv_t = [wp.tile([P, COLS], bf16, tag="wv", name=f"wv{i}") for i in range(NKT)]
        qs_engines = [nc.sync, nc.scalar, nc.gpsimd]
        qi = 0

        def dma_in(out, in_):
            nonlocal qi
            qs_engines[qi % len(qs_engines)].dma_start(out=out, in_=in_)
            qi += 1

        # The k-projection chain consumes (wk[kt], xt[kt] quarter0) in kt
        # order, so interleave those pairs first; wq/wv follow, then the
        # remaining x quarters.  Bias vectors (needed ~20us in) come last.
        sl0 = slice(0, L // 4)
        for kt in range(NKT):
            dma_in(wk_t[kt][:], wk_d[kt])
            dma_in(xt_t[kt][:, sl0], xt_d[kt][:, sl0])
        for kt in range(NKT):
            dma_in(wq_t[kt][:], wq_d[kt])
        for kt in range(NKT):
            dma_in(wv_t[kt][:], wv_d[kt])
        for quarter in range(1, 4):
            sl = slice(quarter * (L // 4), (quarter + 1) * (L // 4))
            for kt in range(NKT):
                dma_in(xt_t[kt][:, sl], xt_d[kt][:, sl])
        wo_t = [wp.tile([P, D], bf16, tag="wo", name=f"wo{i}", bufs=NDT) for i in range(NDT)]
        for dt_i in range(NDT):
            nc.gpsimd.dma_start(out=wo_t[dt_i][:], in_=wo_d[dt_i])
        for mb in range(NMB):
            nc.sync.dma_start(out=bq_sb[:, mb : mb + 1], in_=bq_d[mb])
            nc.sync.dma_start(out=bk_sb[:, mb : mb + 1], in_=bk_d[mb])
        nc.sync.dma_start(out=bvb_sb[:], in_=bvb_d[:])

        # ones row for the tail rec broadcast (rank-1 PE matmul)
        ones_f32 = consts.tile([1, HD], f32, tag="ones_f32")
        nc.vector.memset(ones_f32[:], 1.0)
        ones_sb = consts.tile([1, HD], bf16, tag="ones")
        nc.vector.tensor_copy(ones_sb[:], ones_f32[:])

        # ---- persistent intermediates ----
        # q^T/k^T: partition = qkv col within a 128-block, dims (col_block, token)
        qt_sb = bigs.tile([P, NMB, L], bf16, tag="qt")
        kt_sb = bigs.tile([P, NMB, L], bf16, tag="kt")
        # v natural + ones column: partition = token within block, (kblock, head, hd+1)
        vx_sb = bigs.tile([P, NKB, H_C, HD + 1], bf16, tag="vx")
        nc.vector.memset(vx_sb[:, :, :, HD : HD + 1], 1.0)
        # normalized attention output, transposed: partition = o-dim within a
        # 128-block, dims (dim_block, token)
        ot_sb = bigs.tile([P, NDT, L], bf16, tag="ot")

        # ---- building blocks ----
        def project_qk(mb, w_t_, b_sb, dst, ns):
            ps = psum.tile([P, 1024], f32, tag="big", name="ps_qk")
            for half in range(2):
                for kt in range(NKT):
                    nc.tensor.matmul(
                        ps[:, half * 512 : half * 512 + 512],
                        lhsT=w_t_[kt][:, mb * P : (mb + 1) * P],
                        rhs=xt_t[kt][
                            :, ns * 1024 + half * 512 : ns * 1024 + half * 512 + 512
                        ],
                        start=(kt == 0),
                        stop=(kt == NKT - 1),
                    )
            nc.vector.tensor_scalar_add(
                dst[:, mb, ns * 1024 : (ns + 1) * 1024], ps, b_sb[:, mb : mb + 1]
            )

        def project_v(tb):
            ps = psum.tile([P, 1024], f32, tag="big", name="ps_v")[:, :COLS]
            for kt in range(NKT):
                nc.tensor.matmul(
                    ps,
                    lhsT=xt_t[kt][:, tb * P : (tb + 1) * P],
                    rhs=wv_t[kt][:],
                    start=(kt == 0),
                    stop=(kt == NKT - 1),
                )
            # bias add fused into the psum->SBUF copy (host-broadcast bias)
            nc.vector.tensor_tensor(
                vx_sb[:, tb, :, 0:HD],
                ps.rearrange("p (h d) -> p h d", h=H_C),
                bvb_sb.rearrange("p (h d) -> p h d", h=H_C),
                add,
            )

        y_qs = [nc.gpsimd, nc.sync, nc.scalar]

        def out_proj(tb, y_eng):
            ps = psum.tile([P, 1024], f32, tag="big", name="ps_y")
            for nb in range(2):
                for dt_i in range(NDT):
                    nc.tensor.matmul(
                        ps[:, nb * 512 : nb * 512 + 512],
                        lhsT=ot_sb[:, dt_i, tb * P : (tb + 1) * P],
                        rhs=wo_t[dt_i][:, nb * 512 : (nb + 1) * 512],
                        start=(dt_i == 0),
                        stop=(dt_i == NDT - 1),
                    )
            y_sb = yp.tile([P, 1024], bf16, tag="y", name="y_sb")
            if y_eng == 0:
                nc.scalar.copy(y_sb[:], ps)
            else:
                nc.vector.tensor_copy(y_sb[:], ps)
            y_qs[tb % len(y_qs)].dma_start(
                out=y_d[tb * P : (tb + 1) * P, :], in_=y_sb[:]
            )

        # ---- attention with filler interleaving ----
        # Per key block: scores^T (bf16, K=64, head pairs at PE rows 0/64)
        # -> exp (ACT exact / DVE Schraudolph) -> PV with augmented v,
        # software-pipelined PVLAG blocks behind the score stream.  One
        # filler unit (a projection slice / out_proj) is emitted per key
        # block so the in-order PE queue never drains.
        def attention(mb, qs, filler, last=False):
            qsl = slice(qs * 1024, (qs + 1) * 1024)
            po = [
                psum_o.tile([HD + 1, 1024], f32, tag="po", name=f"po{hh}")
                for hh in range(2)
            ]
            PVLAG = 2
            pending = []

            def emit_pv(kb, p2):
                for hh in range(2):
                    for half in range(2):
                        nc.tensor.matmul(
                            po[hh][:, half * 512 : half * 512 + 512],
                            lhsT=vx_sb[:, kb, 2 * mb + hh, :],
                            rhs=p2[hh][:, half * 512 : half * 512 + 512],
                            start=(kb == 0),
                            stop=(kb == NKB - 1),
                        )

            for kb in range(NKB):
                if filler:
                    filler.popleft()()
                on_dve = kb in DVE_KBS
                p2 = []
                for hh in range(2):
                    off = hh * HD
                    ps = psum.tile([P, 1024], f32, tag="big", name=f"ps_s{hh}")
                    for half in range(2):
                        nc.tensor.matmul(
                            ps[:, half * 512 : half * 512 + 512],
                            lhsT=kt_sb[off : off + HD, mb, kb * P : (kb + 1) * P],
                            rhs=qt_sb[
                                off : off + HD,
                                mb,
                                qs * 1024 + half * 512 : qs * 1024 + half * 512 + 512,
                            ],
                            start=True,
                            stop=True,
                        )
                    p_sb = pp.tile([P, 1024], bf16, tag="p", name=f"p_sb{hh}")
                    if on_dve:
                        nc.vector.tensor_scalar(
                            p_sb[:].bitcast(i16), ps,
                            float(SCHRAUD_A16), float(SCHRAUD_B16), mult, add,
                        )
                    else:
                        nc.scalar.activation(p_sb[:], ps, Exp)
                    p2.append(p_sb)
                pending.append((kb, p2))
                if len(pending) > PVLAG:
                    emit_pv(*pending.pop(0))
            for kb_p, p2_p in pending:
                emit_pv(kb_p, p2_p)

            # normalize: columns of po[0:HD] scaled by 1 / po[HD].  The
            # po-freeing copies run on ScalarE (short queue) so the next
            # slice's PV can claim the po slot quickly.
            for hh in range(2):
                off = hh * HD
                sums_sb = smallp.tile([1, 1024], f32, tag="sums", name="sums_sb")
                nc.scalar.copy(sums_sb[:], po[hh][HD : HD + 1, :])
                o_sb = smallp.tile([HD, 1024], bf16, tag="o_sb", name="o_sb", bufs=2)
                nc.scalar.copy(o_sb[:], po[hh][0:HD, :])
                rec = smallp.tile([1, 1024], f32, tag="rec", name="rec")
                nc.vector.reciprocal_approx_fast(rec[:], sums_sb[:])
                rec_bf = smallp.tile([1, 1024], bf16, tag="rec_bf", name="rec_bf")
                nc.vector.tensor_copy(rec_bf[:], rec[:])
                if last:
                    # tail: the PE is idle here, so broadcast 1/sum across
                    # partitions with a rank-1 matmul instead of the
                    # higher-latency DRAM bounce
                    ps_b = psum.tile([P, 1024], f32, tag="big", name="ps_b")[
                        off : off + HD, :
                    ]
                    for half in range(2):
                        nc.tensor.matmul(
                            ps_b[:, half * 512 : half * 512 + 512],
                            lhsT=ones_sb[:],
                            rhs=rec_bf[:, half * 512 : half * 512 + 512],
                            start=True, stop=True,
                        )
                    nc.vector.tensor_tensor(
                        ot_sb[off : off + HD, mb, qsl], o_sb[:], ps_b, mult
                    )
                else:
                    # partition-broadcast via a DRAM bounce (zero-stride SBUF
                    # APs are rejected; DRAM sources may broadcast)
                    rec_dr = drp.tile([1, 1024], bf16, tag="rec_dr", name="rec_dr")
                    nc.sync.dma_start(out=rec_dr[:], in_=rec_bf[:])
                    pb_sb = smallp.tile([HD, 1024], bf16, tag="pb_sb", name="pb_sb")
                    dr_ap = rec_dr[:]
                    rec_bcast = bass.AP(
                        tensor=dr_ap.tensor,
                        offset=dr_ap.offset,
                        ap=[[0, HD], dr_ap.ap[-1]],
                    )
                    nc.sync.dma_start(out=pb_sb[:], in_=rec_bcast)
                    nc.vector.tensor_tensor(
                        ot_sb[off : off + HD, mb, qsl], o_sb[:], pb_sb[:], mult
                    )

        # ---- schedule ----
        # Minimal prologue: k/q/v for the first 1024-token slice of col-block
        # 0, then attention(0,0) starts while fillers stream the rest of the
        # projections through the attention loop's PE slack.
        project_qk(0, wk_t, bk_sb, kt_sb, 0)
        project_qk(0, wq_t, bq_sb, qt_sb, 0)
        for tb in range(4):
            project_v(tb)

        f00 = deque([
            (lambda t: lambda: project_v(t))(tb) for tb in range(4, 10)
        ])
        f00.insert(2, lambda: project_qk(0, wk_t, bk_sb, kt_sb, 1))
        f00.extend((lambda t: lambda: project_v(t))(tb) for tb in range(10, 16))
        f00.append(lambda: project_qk(0, wq_t, bq_sb, qt_sb, 1))
        attention(0, 0, f00)

        f01 = deque([
            lambda: project_qk(1, wq_t, bq_sb, qt_sb, 0),
            lambda: project_qk(1, wk_t, bk_sb, kt_sb, 0),
            lambda: project_qk(1, wq_t, bq_sb, qt_sb, 1),
            lambda: project_qk(1, wk_t, bk_sb, kt_sb, 1),
        ])
        attention(0, 1, f01)
        attention(1, 0, deque())
        f11 = deque([lambda: None] * 3)
        for i, tb in enumerate(range(0, 8)):
            f11.append((lambda t, e: lambda: out_proj(t, e))(tb, i % 2))
        attention(1, 1, f11, last=True)
        for i, tb in enumerate(range(8, 16)):
            out_proj(tb, y_eng=i % 2)

    nc.finalize()
    return nc


def get_nc():
    global _NC_CACHE
    if _NC_CACHE is None:
        _NC_CACHE = _build_nc()
    return _NC_CACHE


def make_in_maps(x, w_qkv, b_qkv, w_out):
    import ml_dtypes

    bf16 = ml_dtypes.bfloat16
    x = np.asarray(x, dtype=np.float32)
    w_qkv = np.asarray(w_qkv, dtype=np.float32)
    b_qkv = np.asarray(b_qkv, dtype=np.float32)
    w_out = np.asarray(w_out, dtype=np.float32)

    in_maps = []
    for c in range(NCORES):
        b, g = divmod(c, CORES_PER_BATCH)
        cs, ce = g * COLS, (g + 1) * COLS
        xt = np.ascontiguousarray(x[b].T).reshape(NKT, P, L).astype(bf16)
        wq = (w_qkv[:, 0 * D : 1 * D][:, cs:ce] * SCALE).reshape(NKT, P, COLS).astype(bf16)
        wk = np.ascontiguousarray(w_qkv[:, 1 * D : 2 * D][:, cs:ce]).reshape(NKT, P, COLS).astype(bf16)
        wv = np.ascontiguousarray(w_qkv[:, 2 * D : 3 * D][:, cs:ce]).reshape(NKT, P, COLS).astype(bf16)
        bq = np.ascontiguousarray(b_qkv[0 * D : 1 * D][cs:ce] * SCALE).reshape(
            NMB, P, 1
        )
        bk = np.ascontiguousarray(b_qkv[1 * D : 2 * D][cs:ce]).reshape(NMB, P, 1)
        bvb = np.broadcast_to(
            b_qkv[2 * D : 3 * D][cs:ce].astype(np.float32), (P, COLS)
        ).copy()
        wo = np.ascontiguousarray(w_out[cs:ce, :]).reshape(NDT, P, D).astype(bf16)
        in_maps.append(
            dict(xt=xt, wq=wq, wk=wk, wv=wv, bq=bq, bk=bk, bvb=bvb, wo=wo)
        )
    return in_maps


def kernel(x, w_qkv, b_qkv, w_out, b_out, _trace=False, **_kw):
    global LAST_RESULTS
    from concourse.bass_utils import run_bass_kernel_spmd

    nc = get_nc()
    in_maps = make_in_maps(x, w_qkv, b_qkv, w_out)
    res = run_bass_kernel_spmd(nc, in_maps, list(range(NCORES)), trace=_trace, **_kw)
    LAST_RESULTS = res

    b_out = np.asarray(b_out, dtype=np.float32)
    y = np.zeros((B, L, D), dtype=np.float32)
    for c in range(NCORES):
        y[c // CORES_PER_BATCH] += res.results[c]["y"].astype(np.float32)
    y += b_out[None, None, :]
    return y


# revision 13
# speedup vs baseline: 1.2515x; 1.0154x over previous
"""Multi-head attention forward on 8 TRN2 NeuronCores.

Problem: B=2, L=2048, D=1024, H=16, Hd=64 MHA block:
    qkv = x @ w_qkv + b_qkv ; per-head softmax(q k^T / sqrt(Hd)) @ v ; o @ w_out + b_out

Sharding (tensor parallel over heads x batch):
  core c -> batch c//4, heads [4*(c%4), 4*(c%4)+4).
  Each core computes its 4 heads' attention for its batch and a partial
  out-projection (2048, 1024) in bf16. Host sums the 4 partials per batch
  in fp32 and adds b_out.

v4 design notes.  Measured TRN2 facts driving this schedule:
  - The PE streams exactly one psum column per 2.4GHz cycle in every
    mode (bf16/fp8/DoubleRow all measured 216ns serialized for N=512),
    so the bf16 matmul plan below is already at the column floor
    (~394k columns/core).  fp8 DoubleRow halves contraction passes only
    with both operands single-fp8, which costs 2.5-5% output error --
    over the 2e-2 budget.  Everything stays bf16.
  - The HAM activity throttle caps sustained PE duty (~74% long-run,
    50%-duty windows after ~60-90us of saturation); PE idle gaps also
    cost a 1.2GHz p-state ramp on resume.  So the schedule's job is a
    gap-free in-order PE queue, not fewer flops: projection work and
    the out-projection are interleaved into the attention loop as
    "filler" units, one per key block, and the tail is kept minimal.
  - exp splits across ScalarE (exact exp -> bf16) and VectorE
    (one-pass Schraudolph: i16 = s*184.66 + 16250.4 bitcast bf16, ~2%
    systematic, on 3 of 16 key blocks) so neither engine gates the
    attention loop (ACT alone would need 133us > attention PE time).
  - v's bias is added during the psum->SBUF copy against a
    host-broadcast [128,256] bias tile (kills 16 rank-1 PE matmuls);
    v carries a ones column so PV accumulates softmax denominators.
  - y is written bf16 (halves the output DMA) on the GpSimd queue;
    psum->y copies alternate ScalarE/VectorE.
"""

from collections import deque
from contextlib import ExitStack

import numpy as np

B, L, D = 2, 2048, 1024
H, HD = 16, 64
NCORES = 8
CORES_PER_BATCH = 4
H_C = H // CORES_PER_BATCH          # heads per core = 4
COLS = H_C * HD                     # qkv cols per core = 256
P = 128
NKT = D // P                        # 8 contraction tiles over D
NKB = L // P                        # 16 key/token blocks of 128
NMB = COLS // P                     # 2 col-blocks of the per-core qkv slice
NDT = COLS // P                     # 2 contraction tiles over per-core o dims
NS2 = L // 1024                     # 2 1024-token slices
SCALE = 1.0 / np.sqrt(np.float32(HD))
LOG2E = 1.4426950408889634
# bf16-bit-trick exp on (pre-scaled) scores: i16 = s*A16 + B16, bitcast bf16
SCHRAUD_A16 = 128.0 * LOG2E
SCHRAUD_B16 = 16256.0 - 5.58

_NC_CACHE = None
LAST_RESULTS = None


def _build_nc():
    import os
    import concourse.bass as bass
    import concourse.tile as tile
    from concourse import bacc, mybir

    f32 = mybir.dt.float32
    bf16 = mybir.dt.bfloat16
    i16 = mybir.dt.int16
    Exp = mybir.ActivationFunctionType.Exp
    mult = mybir.AluOpType.mult
    add = mybir.AluOpType.add

    # score blocks whose exp runs on VectorE via the bf16 bit trick
    DVE_KBS = frozenset(
        int(t) for t in os.environ.get("KDVE", "3,8,13").split(",") if t != ""
    )

    nc = bacc.Bacc(None, target_bir_lowering=False)

    xt_d = nc.declare_dram_parameter("xt", [NKT, P, L], bf16, isOutput=False)
    wq_d = nc.declare_dram_parameter("wq", [NKT, P, COLS], bf16, isOutput=False)
    wk_d = nc.declare_dram_parameter("wk", [NKT, P, COLS], bf16, isOutput=False)
    wv_d = nc.declare_dram_parameter("wv", [NKT, P, COLS], bf16, isOutput=False)
    bq_d = nc.declare_dram_parameter("bq", [NMB, P, 1], f32, isOutput=False)
    bk_d = nc.declare_dram_parameter("bk", [NMB, P, 1], f32, isOutput=False)
    bvb_d = nc.declare_dram_parameter("bvb", [P, COLS], f32, isOutput=False)
    wo_d = nc.declare_dram_parameter("wo", [NDT, P, D], bf16, isOutput=False)
    y_d = nc.declare_dram_parameter("y", [L, D], bf16, isOutput=True)

    with tile.TileContext(nc) as tc, ExitStack() as ctx, nc.allow_low_precision(
        "bf16 matmul operands; accumulation stays fp32 in PSUM"
    ):
        consts = ctx.enter_context(tc.tile_pool(name="consts", bufs=1))
        xtp = ctx.enter_context(tc.tile_pool(name="xtp", bufs=NKT))
        wp = ctx.enter_context(tc.tile_pool(name="wp", bufs=NKT))
        bigs = ctx.enter_context(tc.tile_pool(name="bigs", bufs=1))
        pp = ctx.enter_context(tc.tile_pool(name="pp", bufs=6))
        yp = ctx.enter_context(tc.tile_pool(name="yp", bufs=3))
        smallp = ctx.enter_context(tc.tile_pool(name="smallp", bufs=2))
        drp = ctx.enter_context(tc.tile_pool(name="drp", bufs=4, space="DRAM"))
        # PSUM budget (8 banks): big (128x1024 f32, 2 banks) x2 bufs = 4,
        # po (65x1024, 2 banks) x2 bufs = 4.
        psum = ctx.enter_context(tc.tile_pool(name="psum", bufs=2, space="PSUM"))
        psum_o = ctx.enter_context(tc.tile_pool(name="psum_o", bufs=2, space="PSUM"))

        # ---- constants ----
        bq_sb = consts.tile([P, NMB], f32, tag="bq")
        bk_sb = consts.tile([P, NMB], f32, tag="bk")
        bvb_sb = consts.tile([P, COLS], f32, tag="bvb")

        # ---- stream in x^T and weights as per-k-tile tiles ----
        xt_t = [xtp.tile([P, L], bf16, tag="xt", name=f"xt{i}") for i in range(NKT)]
        wq_t = [wp.tile([P, COLS], bf16, tag="wq", name=f"wq{i}") for i in range(NKT)]
        wk_t = [wp.tile([P, COLS], bf16, tag="wk", name=f"wk{i}") for i in range(NKT)]
        wv_t = [wp.tile([P, COLS], bf16, tag="wv", name=f"wv{i}") for i in range(NKT)]
        qs_engines = [nc.sync, nc.scalar, nc.gpsimd]
        qi = 0

        def dma_in(out, in_):
            nonlocal qi
            qs_engines[qi % len(qs_engines)].dma_start(out=out, in_=in_)
            qi += 1

        # The k-projection chain consumes (wk[kt], xt[kt] quarter0) in kt
        # order, so interleave those pairs first; wq/wv follow, then the
        # remaining x quarters.  Bias vectors (needed ~20us in) come last.
        sl0 = slice(0, L // 4)
        for kt in range(NKT):
            dma_in(wk_t[kt][:], wk_d[kt])
            dma_in(xt_t[kt][:, sl0], xt_d[kt][:, sl0])
        for kt in range(NKT):
            dma_in(wq_t[kt][:], wq_d[kt])
        for kt in range(NKT):
            dma_in(wv_t[kt][:], wv_d[kt])
        for quarter in range(1, 4):
            sl = slice(quarter * (L // 4), (quarter + 1) * (L // 4))
            for kt in range(NKT):
                dma_in(xt_t[kt][:, sl], xt_d[kt][:, sl])
        wo_t = [wp.tile([P, D], bf16, tag="wo", name=f"wo{i}", bufs=NDT) for i in range(NDT)]
        for dt_i in range(NDT):
            nc.gpsimd.dma_start(out=wo_t[dt_i][:], in_=wo_d[dt_i])
        for mb in range(NMB):
            nc.sync.dma_start(out=bq_sb[:, mb : mb + 1], in_=bq_d[mb])
            nc.sync.dma_start(out=bk_sb[:, mb : mb + 1], in_=bk_d[mb])
        nc.sync.dma_start(out=bvb_sb[:], in_=bvb_d[:])

        # ones row for the tail rec broadcast (rank-1 PE matmul)
        ones_f32 = consts.tile([1, HD], f32, tag="ones_f32")
        nc.vector.memset(ones_f32[:], 1.0)
        ones_sb = consts.tile([1, HD], bf16, tag="ones")
        nc.vector.tensor_copy(ones_sb[:], ones_f32[:])

        # ---- persistent intermediates ----
        # q^T/k^T: partition = qkv col within a 128-block, dims (col_block, token)
        qt_sb = bigs.tile([P, NMB, L], bf16, tag="qt")
        kt_sb = bigs.tile([P, NMB, L], bf16, tag="kt")
        # v natural + ones column: partition = token within block, (kblock, head, hd+1)
        vx_sb = bigs.tile([P, NKB, H_C, HD + 1], bf16, tag="vx")
        nc.vector.memset(vx_sb[:, :, :, HD : HD + 1], 1.0)
        # normalized attention output, transposed: partition = o-dim within a
        # 128-block, dims (dim_block, token)
        ot_sb = bigs.tile([P, NDT, L], bf16, tag="ot")

        # ---- building blocks ----
        def project_qk(mb, w_t_, b_sb, dst, ns):
            ps = psum.tile([P, 1024], f32, tag="big", name="ps_qk")
            for half in range(2):
                for kt in range(NKT):
                    nc.tensor.matmul(
                        ps[:, half * 512 : half * 512 + 512],
                        lhsT=w_t_[kt][:, mb * P : (mb + 1) * P],
                        rhs=xt_t[kt][
                            :, ns * 1024 + half * 512 : ns * 1024 + half * 512 + 512
                        ],
                        start=(kt == 0),
                        stop=(kt == NKT - 1),
                    )
            nc.vector.tensor_scalar_add(
                dst[:, mb, ns * 1024 : (ns + 1) * 1024], ps, b_sb[:, mb : mb + 1]
            )

        def project_v(tb):
            ps = psum.tile([P, 1024], f32, tag="big", name="ps_v")[:, :COLS]
            for kt in range(NKT):
                nc.tensor.matmul(
                    ps,
                    lhsT=xt_t[kt][:, tb * P : (tb + 1) * P],
                    rhs=wv_t[kt][:],
                    start=(kt == 0),
                    stop=(kt == NKT - 1),
                )
            # bias add fused into the psum->SBUF copy (host-broadcast bias)
            nc.vector.tensor_tensor(
                vx_sb[:, tb, :, 0:HD],
                ps.rearrange("p (h d) -> p h d", h=H_C),
                bvb_sb.rearrange("p (h d) -> p h d", h=H_C),
                add,
            )

        y_qs = [nc.gpsimd, nc.sync, nc.scalar]

        def out_proj(tb, y_eng):
            ps = psum.tile([P, 1024], f32, tag="big", name="ps_y")
            for nb in range(2):
                for dt_i in range(NDT):
                    nc.tensor.matmul(
                        ps[:, nb * 512 : nb * 512 + 512],
                        lhsT=ot_sb[:, dt_i, tb * P : (tb + 1) * P],
                        rhs=wo_t[dt_i][:, nb * 512 : (nb + 1) * 512],
                        start=(dt_i == 0),
                        stop=(dt_i == NDT - 1),
                    )
            y_sb = yp.tile([P, 1024], bf16, tag="y", name="y_sb")
            if y_eng == 2:
                # tail: halve the drain latency by splitting copy + DMA
                nc.scalar.copy(y_sb[:, 0:512], ps[:, 0:512])
                nc.vector.tensor_copy(y_sb[:, 512:1024], ps[:, 512:1024])
                y_qs[tb % len(y_qs)].dma_start(
                    out=y_d[tb * P : (tb + 1) * P, 0:512], in_=y_sb[:, 0:512]
                )
                y_qs[(tb + 1) % len(y_qs)].dma_start(
                    out=y_d[tb * P : (tb + 1) * P, 512:1024], in_=y_sb[:, 512:1024]
                )
                return
            if y_eng == 0:
                nc.scalar.copy(y_sb[:], ps)
            else:
                nc.vector.tensor_copy(y_sb[:], ps)
            y_qs[tb % len(y_qs)].dma_start(
                out=y_d[tb * P : (tb + 1) * P, :], in_=y_sb[:]
            )

        # ---- attention with filler interleaving ----
        # Per key block: scores^T (bf16, K=64, head pairs at PE rows 0/64)
        # -> exp (ACT exact / DVE Schraudolph) -> PV with augmented v,
        # software-pipelined PVLAG blocks behind the score stream.  One
        # filler unit (a projection slice / out_proj) is emitted per key
        # block so the in-order PE queue never drains.
        def attention(mb, qs, filler, last=False):
            qsl = slice(qs * 1024, (qs + 1) * 1024)
            po = [
                psum_o.tile([HD + 1, 1024], f32, tag="po", name=f"po{hh}")
                for hh in range(2)
            ]
            PVLAG = 2
            pending = []

            def emit_pv(kb, p2):
                for hh in range(2):
                    for half in range(2):
                        nc.tensor.matmul(
                            po[hh][:, half * 512 : half * 512 + 512],
                            lhsT=vx_sb[:, kb, 2 * mb + hh, :],
                            rhs=p2[hh][:, half * 512 : half * 512 + 512],
                            start=(kb == 0),
                            stop=(kb == NKB - 1),
                        )

            for kb in range(NKB):
                if filler:
                    filler.popleft()()
                on_dve = kb in DVE_KBS
                p2 = []
                for hh in range(2):
                    off = hh * HD
                    ps = psum.tile([P, 1024], f32, tag="big", name=f"ps_s{hh}")
                    for half in range(2):
                        nc.tensor.matmul(
                            ps[:, half * 512 : half * 512 + 512],
                            lhsT=kt_sb[off : off + HD, mb, kb * P : (kb + 1) * P],
                            rhs=qt_sb[
                                off : off + HD,
                                mb,
                                qs * 1024 + half * 512 : qs * 1024 + half * 512 + 512,
                            ],
                            start=True,
                            stop=True,
                        )
                    p_sb = pp.tile([P, 1024], bf16, tag="p", name=f"p_sb{hh}")
                    if on_dve:
                        nc.vector.tensor_scalar(
                            p_sb[:].bitcast(i16), ps,
                            float(SCHRAUD_A16), float(SCHRAUD_B16), mult, add,
                        )
                    else:
                        nc.scalar.activation(p_sb[:], ps, Exp)
                    p2.append(p_sb)
                pending.append((kb, p2))
                if len(pending) > PVLAG:
                    emit_pv(*pending.pop(0))
            for kb_p, p2_p in pending:
                emit_pv(kb_p, p2_p)

            # normalize: columns of po[0:HD] scaled by 1 / po[HD].  The
            # po-freeing copies run on ScalarE (short queue) so the next
            # slice's PV can claim the po slot quickly.
            for hh in range(2):
                off = hh * HD
                sums_sb = smallp.tile([1, 1024], f32, tag="sums", name="sums_sb")
                o_sb = smallp.tile([HD, 1024], bf16, tag="o_sb", name="o_sb", bufs=2)
                if hh == 0:
                    nc.scalar.copy(sums_sb[:], po[hh][HD : HD + 1, :])
                    nc.scalar.copy(o_sb[:], po[hh][0:HD, :])
                else:
                    nc.vector.tensor_copy(sums_sb[:], po[hh][HD : HD + 1, :])
                    nc.vector.tensor_copy(o_sb[:], po[hh][0:HD, :])
                rec = smallp.tile([1, 1024], f32, tag="rec", name="rec")
                nc.vector.reciprocal_approx_fast(rec[:], sums_sb[:])
                rec_bf = smallp.tile([1, 1024], bf16, tag="rec_bf", name="rec_bf")
                nc.vector.tensor_copy(rec_bf[:], rec[:])
                if last:
                    # tail: the PE is idle here, so broadcast 1/sum across
                    # partitions with a rank-1 matmul instead of the
                    # higher-latency DRAM bounce
                    ps_b = psum.tile([P, 1024], f32, tag="big", name="ps_b")[
                        off : off + HD, :
                    ]
                    for half in range(2):
                        nc.tensor.matmul(
                            ps_b[:, half * 512 : half * 512 + 512],
                            lhsT=ones_sb[:],
                            rhs=rec_bf[:, half * 512 : half * 512 + 512],
                            start=True, stop=True,
                        )
                    nc.vector.tensor_tensor(
                        ot_sb[off : off + HD, mb, qsl], o_sb[:], ps_b, mult
                    )
                else:
                    # partition-broadcast via a DRAM bounce (zero-stride SBUF
                    # APs are rejected; DRAM sources may broadcast)
                    rec_dr = drp.tile([1, 1024], bf16, tag="rec_dr", name="rec_dr")
                    nc.sync.dma_start(out=rec_dr[:], in_=rec_bf[:])
                    pb_sb = smallp.tile([HD, 1024], bf16, tag="pb_sb", name="pb_sb")
                    dr_ap = rec_dr[:]
                    rec_bcast = bass.AP(
                        tensor=dr_ap.tensor,
                        offset=dr_ap.offset,
                        ap=[[0, HD], dr_ap.ap[-1]],
                    )
                    nc.sync.dma_start(out=pb_sb[:], in_=rec_bcast)
                    nc.vector.tensor_tensor(
                        ot_sb[off : off + HD, mb, qsl], o_sb[:], pb_sb[:], mult
                    )

        # ---- schedule ----
        # Minimal prologue: k/q/v for the first 1024-token slice of col-block
        # 0, then attention(0,0) starts while fillers stream the rest of the
        # projections through the attention loop's PE slack.
        project_qk(0, wk_t, bk_sb, kt_sb, 0)
        project_qk(0, wq_t, bq_sb, qt_sb, 0)
        for tb in range(4):
            project_v(tb)

        f00 = deque([
            (lambda t: lambda: project_v(t))(tb) for tb in range(4, 10)
        ])
        f00.insert(2, lambda: project_qk(0, wk_t, bk_sb, kt_sb, 1))
        f00.extend((lambda t: lambda: project_v(t))(tb) for tb in range(10, 16))
        f00.append(lambda: project_qk(0, wq_t, bq_sb, qt_sb, 1))
        attention(0, 0, f00)

        f01 = deque([
            lambda: project_qk(1, wq_t, bq_sb, qt_sb, 0),
            lambda: project_qk(1, wk_t, bk_sb, kt_sb, 0),
            lambda: project_qk(1, wq_t, bq_sb, qt_sb, 1),
            lambda: project_qk(1, wk_t, bk_sb, kt_sb, 1),
        ])
        attention(0, 1, f01)
        attention(1, 0, deque())
        f11 = deque([lambda: None] * 3)
        for i, tb in enumerate(range(0, 8)):
            f11.append((lambda t, e: lambda: out_proj(t, e))(tb, i % 2))
        attention(1, 1, f11, last=True)
        for tb in range(8, 16):
            out_proj(tb, y_eng=2)

    nc.finalize()
    return nc


def get_nc():
    global _NC_CACHE
    if _NC_CACHE is None:
        _NC_CACHE = _build_nc()
    return _NC_CACHE


def make_in_maps(x, w_qkv, b_qkv, w_out):
    import ml_dtypes

    bf16 = ml_dtypes.bfloat16
    x = np.asarray(x, dtype=np.float32)
    w_qkv = np.asarray(w_qkv, dtype=np.float32)
    b_qkv = np.asarray(b_qkv, dtype=np.float32)
    w_out = np.asarray(w_out, dtype=np.float32)

    in_maps = []
    for c in range(NCORES):
        b, g = divmod(c, CORES_PER_BATCH)
        cs, ce = g * COLS, (g + 1) * COLS
        xt = np.ascontiguousarray(x[b].T).reshape(NKT, P, L).astype(bf16)
        wq = (w_qkv[:, 0 * D : 1 * D][:, cs:ce] * SCALE).reshape(NKT, P, COLS).astype(bf16)
        wk = np.ascontiguousarray(w_qkv[:, 1 * D : 2 * D][:, cs:ce]).reshape(NKT, P, COLS).astype(bf16)
        wv = np.ascontiguousarray(w_qkv[:, 2 * D : 3 * D][:, cs:ce]).reshape(NKT, P, COLS).astype(bf16)
        bq = np.ascontiguousarray(b_qkv[0 * D : 1 * D][cs:ce] * SCALE).reshape(
            NMB, P, 1
        )
        bk = np.ascontiguousarray(b_qkv[1 * D : 2 * D][cs:ce]).reshape(NMB, P, 1)
        bvb = np.broadcast_to(
            b_qkv[2 * D : 3 * D][cs:ce].astype(np.float32), (P, COLS)
        ).copy()
        wo = np.ascontiguousarray(w_out[cs:ce, :]).reshape(NDT, P, D).astype(bf16)
        in_maps.append(
            dict(xt=xt, wq=wq, wk=wk, wv=wv, bq=bq, bk=bk, bvb=bvb, wo=wo)
        )
    return in_maps


def kernel(x, w_qkv, b_qkv, w_out, b_out, _trace=False, **_kw):
    global LAST_RESULTS
    from concourse.bass_utils import run_bass_kernel_spmd

    nc = get_nc()
    in_maps = make_in_maps(x, w_qkv, b_qkv, w_out)
    res = run_bass_kernel_spmd(nc, in_maps, list(range(NCORES)), trace=_trace, **_kw)
    LAST_RESULTS = res

    b_out = np.asarray(b_out, dtype=np.float32)
    y = np.zeros((B, L, D), dtype=np.float32)
    for c in range(NCORES):
        y[c // CORES_PER_BATCH] += res.results[c]["y"].astype(np.float32)
    y += b_out[None, None, :]
    return y
